# revision 2
# baseline (speedup 1.0000x reference)
"""Dropless MoE FFN (router + top-2 dispatch + per-expert MLP + combine) on
8 Trainium2 NeuronCores.

Strategy (expert parallelism, per the sharding hint):
  - Router (softmax + top-2) runs on host in fp32 — it is ~0.02% of the
    FLOPs and IS the token dispatch: each of the 8 cores owns one expert
    and receives only the tokens routed to it (gather on host replaces the
    device all-to-all; full inputs in / full output out per the contract).
  - Each core computes y = (gelu_tanh(x_e @ w1_e) @ w2_e) * combine_weight
    for its ~1k routed tokens, entirely in bf16 matmuls (fp32 PSUM
    accumulation), weights read from HBM exactly once.
  - Host scatter-adds the two scaled expert outputs per token (combine).

Device kernel layout per core (C = token capacity, padded to 128):
  GEMM1 (hT = w1.T-slices @ xT):  out[f_tile, tokens] so no transpose is
  ever needed between the two GEMMs; GELU applied PSUM->SBUF on ScalarE,
  producing bf16 hT resident in SBUF.  GEMM2 accumulates over all 32
  f-chunks into PSUM per 128-token tile, scaled by the per-token combine
  weight during the PSUM->SBUF copy (ScalarE, per-partition scale).
"""

import sys

for _p in ("/opt/trn_rl_repo",):
    if _p not in sys.path:
        sys.path.insert(0, _p)

import numpy as np
import ml_dtypes

BF16 = ml_dtypes.bfloat16

D_MODEL = 1024
D_FFN = 4096
N_EXPERTS = 8
TOP_K = 2
N_CORES = 8
P = 128                 # SBUF/PSUM partitions
KC = D_MODEL // P       # 8 contraction chunks for GEMM1
FC = D_FFN // P         # 32 f-chunks (contraction chunks for GEMM2)
MB = 4                  # w1 streamed in 4 blocks of 1024 f-columns

_kernel_cache: dict[int, object] = {}


def _token_groups(C):
    """Split C token columns into <=512-wide groups, multiples of 128."""
    n_g = -(-C // 512)
    base = (C // n_g) // P * P
    sizes = [base] * n_g
    rem = C - base * n_g
    i = 0
    while rem > 0:
        sizes[i] += P
        rem -= P
        i += 1
    offs = [0]
    for s in sizes[:-1]:
        offs.append(offs[-1] + s)
    return list(zip(offs, sizes))


def _build(C):
    import concourse.bass as bass
    import concourse.mybir as mybir
    import concourse.tile as tile
    from concourse import bacc

    dt = mybir.dt
    AF = mybir.ActivationFunctionType
    TT = C // P
    groups = _token_groups(C)

    nc = bacc.Bacc("TRN2", target_bir_lowering=False, debug=False,
                   num_devices=N_CORES)
    xt_d = nc.dram_tensor("xt", [KC, P, C], dt.bfloat16,
                          kind="ExternalInput").ap()
    w1_d = nc.dram_tensor("w1", [MB, KC, P, KC * P], dt.bfloat16,
                          kind="ExternalInput").ap()
    w2_d = nc.dram_tensor("w2", [FC, P, D_MODEL], dt.bfloat16,
                          kind="ExternalInput").ap()
    wt_d = nc.dram_tensor("wt", [P, TT], dt.float32,
                          kind="ExternalInput").ap()
    y_d = nc.dram_tensor("y", [TT, P, D_MODEL], dt.float32,
                         kind="ExternalOutput").ap()

    with tile.TileContext(nc) as tc:
        with (
            tc.tile_pool(name="xt", bufs=KC) as xt_pool,
            tc.tile_pool(name="w1", bufs=12) as w1_pool,
            tc.tile_pool(name="w2", bufs=FC) as w2_pool,
            tc.tile_pool(name="ht", bufs=FC) as ht_pool,
            tc.tile_pool(name="yo", bufs=2) as y_pool,
            tc.tile_pool(name="wt", bufs=1) as wt_pool,
            tc.tile_pool(name="ps1", bufs=4, space=bass.MemorySpace.PSUM) as ps1_pool,
            tc.tile_pool(name="ps2", bufs=4, space=bass.MemorySpace.PSUM) as ps2_pool,
        ):
            wt_t = wt_pool.tile([P, TT], dt.float32, tag="wt")
            nc.sync.dma_start(wt_t[:], wt_d[:])

            xt_t = []
            for kc in range(KC):
                t = xt_pool.tile([P, C], dt.bfloat16, tag="xt")
                nc.sync.dma_start(t[:], xt_d[kc])
                xt_t.append(t)

            # ---- GEMM1: hT[m*128+p, t] = sum_k w1[k, f] * x[t, k], + GELU
            ht_t = []
            w2_t = []
            w1_t = None
            for m in range(FC):
                mb, mi = divmod(m, KC)
                if mi == 0:
                    w1_t = [w1_pool.tile([P, KC * P], dt.bfloat16, tag="w1",
                                         name=f"w1_{mb}_{kc}")
                            for kc in range(KC)]
                    for kc in range(KC):
                        nc.sync.dma_start(w1_t[kc][:], w1_d[mb, kc])
                ps = [ps1_pool.tile([P, 512], dt.float32, tag="ps1",
                                    name=f"ps1_{m}_{g}")
                      for g in range(len(groups))]
                for kc in range(KC):
                    lhsT = w1_t[kc][:, mi * P:(mi + 1) * P]
                    for g, (off, sz) in enumerate(groups):
                        nc.tensor.matmul(ps[g][:, :sz], lhsT,
                                         xt_t[kc][:, off:off + sz],
                                         start=(kc == 0), stop=(kc == KC - 1))
                ht = ht_pool.tile([P, C], dt.bfloat16, tag="ht")
                for g, (off, sz) in enumerate(groups):
                    nc.scalar.activation(ht[:, off:off + sz], ps[g][:, :sz],
                                         AF.Gelu_apprx_tanh)
                ht_t.append(ht)
                # pace one w2 chunk prefetch per GEMM1 iteration
                w2t = w2_pool.tile([P, D_MODEL], dt.bfloat16, tag="w2")
                nc.sync.dma_start(w2t[:], w2_d[m])
                w2_t.append(w2t)

            # ---- GEMM2: y[t, d] = (sum_f h[t, f] * w2[f, d]) * wt[t]
            for tt in range(TT):
                ps_h = [ps2_pool.tile([P, 512], dt.float32, tag="ps2",
                                      name=f"ps2_{tt}_{h}")
                        for h in range(2)]
                for fc in range(FC):
                    lhsT = ht_t[fc][:, tt * P:(tt + 1) * P]
                    for h in range(2):
                        nc.tensor.matmul(ps_h[h], lhsT,
                                         w2_t[fc][:, h * 512:(h + 1) * 512],
                                         start=(fc == 0), stop=(fc == FC - 1))
                y_t = y_pool.tile([P, D_MODEL], dt.float32, tag="yo")
                for h in range(2):
                    nc.scalar.activation(y_t[:, h * 512:(h + 1) * 512],
                                         ps_h[h][:], AF.Copy,
                                         scale=wt_t[:, tt:tt + 1])
                nc.sync.dma_start(y_d[tt], y_t[:])

    nc.compile()
    return nc


def _route(x, router_w):
    """Replicate the reference router math (jax on CPU, fp32)."""
    import jax
    import jax.numpy as jnp

    with jax.default_device(jax.devices("cpu")[0]):
        xt = jnp.asarray(np.asarray(x, np.float32)).reshape(-1, D_MODEL)
        logits = xt @ jnp.asarray(np.asarray(router_w, np.float32))
        probs = jax.nn.softmax(logits, axis=-1)
        top_p, top_i = jax.lax.top_k(probs, TOP_K)
    return np.asarray(top_p), np.asarray(top_i)


def _run(x, router_w, w1, w2, trace=False):
    from concourse import bass_utils

    x = np.asarray(x, np.float32)
    w1 = np.asarray(w1, np.float32)
    w2 = np.asarray(w2, np.float32)
    B, S, _ = x.shape
    T = B * S
    xt = x.reshape(T, D_MODEL)

    top_p, top_i = _route(x, router_w)

    idxs, wts = [], []
    maxn = 0
    for e in range(N_EXPERTS):
        hit = top_i == e                       # [T, K]
        sel = hit.any(axis=1)
        idx = np.nonzero(sel)[0]
        w = (top_p * hit).sum(axis=1)[sel]     # combine weight per routed token
        idxs.append(idx)
        wts.append(w.astype(np.float32))
        maxn = max(maxn, len(idx))

    C = max(((maxn + P - 1) // P) * P, 2 * P)
    nc = _kernel_cache.get(C)
    if nc is None:
        nc = _build(C)
        _kernel_cache[C] = nc
    TT = C // P

    in_maps = []
    for e in range(N_EXPERTS):
        n = len(idxs[e])
        xg = np.zeros((C, D_MODEL), np.float32)
        xg[:n] = xt[idxs[e]]
        xtb = np.ascontiguousarray(xg.T).astype(BF16).reshape(KC, P, C)
        w1b = (w1[e].astype(BF16)
               .reshape(KC, P, MB, KC * P).transpose(2, 0, 1, 3))
        w1b = np.ascontiguousarray(w1b)
        w2b = np.ascontiguousarray(w2[e].astype(BF16).reshape(FC, P, D_MODEL))
        wpad = np.zeros(C, np.float32)
        wpad[:n] = wts[e]
        wtb = np.ascontiguousarray(wpad.reshape(TT, P).T)
        in_maps.append({"xt": xtb, "w1": w1b, "w2": w2b, "wt": wtb})

    res = bass_utils.run_bass_kernel_spmd(
        nc, in_maps, core_ids=list(range(N_CORES)), trace=trace)

    out = np.zeros((T, D_MODEL), np.float32)
    for e in range(N_EXPERTS):
        n = len(idxs[e])
        y = np.asarray(res.results[e]["y"], np.float32).reshape(C, D_MODEL)
        np.add.at(out, idxs[e], y[:n])
    return out.reshape(B, S, D_MODEL), res


def kernel(**inputs):
    out, _ = _run(inputs["x"], inputs["router_w"], inputs["w1"], inputs["w2"])
    return out
